# revision 9
# baseline (speedup 1.0000x reference)
"""DWTFM fused kernel for Trainium2 (Bass/Tile), 8-core data parallel.

Math: the reference computes LL of dwt(x0), LH/HL/HH of dwt(x1), then idwt.
Algebraically this collapses to a local 2x2 stencil:

    out[2i+r, 2j+s] = x1[2i+r, 2j+s] + 0.25 * sum_{r',s'} (x0 - x1)[2i+r', 2j+s']

i.e. out = x1 + upsample2x2(blockmean2x2(x0 - x1)), independently per (b, c).

Layout: per core, the [2, 3, 512, 512] shard is viewed as [1536, 1024] where
each row is one "block-row" = two consecutive image rows concatenated
([even_row(512) | odd_row(512)]). 2x2 blocks never straddle rows of this view.
"""

import numpy as np

_B, _C, _H, _W = 16, 3, 512, 512
_NCORES = 8
_BPC = _B // _NCORES          # batch entries per core
_ROWS = _BPC * _C * _H // 2   # 1536 block-rows per core
_COLS = 2 * _W                # 1024
_P = 128                      # partitions per tile
_NT = _ROWS // _P             # 12 chunks per core

TRACE = False
LAST_RESULTS = None

_built = None


def _build():
    import concourse.bacc as bacc
    import concourse.mybir as mybir
    from concourse.tile import TileContext

    f32 = mybir.dt.float32
    W = _W
    J = W // 2  # 256 blocks per image row

    nc = bacc.Bacc("TRN2", target_bir_lowering=False)
    x0 = nc.dram_tensor("x0", [_ROWS, _COLS], f32, kind="ExternalInput").ap()
    x1 = nc.dram_tensor("x1", [_ROWS, _COLS], f32, kind="ExternalInput").ap()
    y = nc.dram_tensor("y", [_ROWS, _COLS], f32, kind="ExternalOutput").ap()

    with TileContext(nc) as tc:
        with tc.tile_pool(name="pool", bufs=3) as pool:
            for k in range(_NT):
                r = k * _P
                t0 = pool.tile([_P, _COLS], f32, name="t0")
                t1 = pool.tile([_P, _COLS], f32, name="t1")
                nc.sync.dma_start(out=t0[:], in_=x0[r : r + _P, :])
                nc.sync.dma_start(out=t1[:], in_=x1[r : r + _P, :])

                # Vertical pair sums per input (each DVE op waits on only one
                # DMA - keeps per-instruction sync-wait counts low), then
                # subtract: v = (x0_even + x0_odd) - (x1_even + x1_odd).
                a = pool.tile([_P, W], f32, name="a")
                nc.vector.tensor_add(out=a[:], in0=t0[:, 0:W], in1=t0[:, W : 2 * W])
                b = pool.tile([_P, W], f32, name="b")
                nc.vector.tensor_add(out=b[:], in0=t1[:, 0:W], in1=t1[:, W : 2 * W])
                v = pool.tile([_P, W], f32, name="v")
                nc.vector.tensor_sub(out=v[:], in0=a[:], in1=b[:])
                # m[p, j] = v[p, 2j] + v[p, 2j+1]  (horizontal pair sum)
                m = pool.tile([_P, J], f32, name="m")
                v2 = v.rearrange("p (j s) -> p j s", s=2)
                nc.vector.tensor_add(out=m[:], in0=v2[:, :, 0], in1=v2[:, :, 1])

                # mu[p, 2j+s] = 0.25 * m[p, j]  (horizontal upsample + scale)
                # on the otherwise-idle Scalar engine.
                mu = pool.tile([_P, W], f32, name="mu")
                mu3 = mu.rearrange("p (j s) -> p j s", s=2)
                mb = m.unsqueeze(2).broadcast_to([_P, J, 2])
                nc.scalar.activation(
                    mu3[:], mb, mybir.ActivationFunctionType.Copy, scale=0.25
                )

                # y = x1 + mu broadcast over the row-of-pair axis; single
                # 3D TensorTensor so yt has one writer (keeps the store
                # DMA's sync-wait count within ISA limits).
                yt = pool.tile([_P, _COLS], f32, name="yt")
                y3 = yt.rearrange("p (r2 w) -> p r2 w", r2=2)
                x3 = t1.rearrange("p (r2 w) -> p r2 w", r2=2)
                mub = mu.unsqueeze(1).broadcast_to([_P, 2, W])
                nc.vector.tensor_add(out=y3[:], in0=x3[:], in1=mub)
                nc.sync.dma_start(out=y[r : r + _P, :], in_=yt[:])
    nc.compile()
    return nc


def _get_nc():
    global _built
    if _built is None:
        _built = _build()
    return _built


_runner = None


def _get_runner():
    """Build (once) a jitted 8-core shard_map callable wrapping the Bass NEFF.

    Mirrors concourse.bass2jax.run_bass_via_pjrt but caches the jitted
    function so repeated kernel() calls skip retracing/recompilation, and
    skips output-buffer donation so on-device operands can be reused.
    """
    global _runner
    if _runner is not None:
        return _runner

    import jax
    import concourse.mybir as mybir
    from concourse import bass2jax
    from jax.experimental.shard_map import shard_map
    from jax.sharding import Mesh, PartitionSpec

    nc = _get_nc()
    bass2jax.install_neuronx_cc_hook()

    partition_name = (
        nc.partition_id_tensor.name if nc.partition_id_tensor else None
    )
    in_names = []
    out_names = []
    out_avals = []
    for alloc in nc.m.functions[0].allocations:
        if not isinstance(alloc, mybir.MemoryLocationSet):
            continue
        name = alloc.memorylocations[0].name
        if alloc.kind == "ExternalInput":
            if name != partition_name:
                in_names.append(name)
        elif alloc.kind == "ExternalOutput":
            out_names.append(name)
            out_avals.append(
                jax.core.ShapedArray(
                    tuple(alloc.tensor_shape), mybir.dt.np(alloc.dtype)
                )
            )
    assert in_names == ["x0", "x1"] and out_names == ["y"], (in_names, out_names)
    all_in_names = tuple(in_names + out_names)
    if partition_name is not None:
        all_in_names = all_in_names + (partition_name,)

    def _body(*args):
        operands = list(args)
        if partition_name is not None:
            operands.append(bass2jax.partition_id_tensor())
        outs = bass2jax._bass_exec_p.bind(
            *operands,
            out_avals=tuple(out_avals),
            in_names=all_in_names,
            out_names=tuple(out_names),
            lowering_input_output_aliases=(),
            sim_require_finite=True,
            sim_require_nnan=True,
            nc=nc,
        )
        return tuple(outs)

    devices = jax.devices()[:_NCORES]
    mesh = Mesh(np.asarray(devices), ("core",))
    n_args = len(in_names) + len(out_names)
    fn = jax.jit(
        shard_map(
            _body,
            mesh=mesh,
            in_specs=(PartitionSpec("core"),) * n_args,
            out_specs=(PartitionSpec("core"),) * len(out_names),
            check_rep=False,
        ),
        keep_unused=True,
    )
    zeros = jax.device_put(
        np.zeros((_NCORES * _ROWS, _COLS), np.float32),
        jax.sharding.NamedSharding(mesh, PartitionSpec("core")),
    )
    _runner = (fn, zeros, mesh)
    return _runner


def kernel(x0: np.ndarray, x1: np.ndarray) -> np.ndarray:
    fn, zeros, _mesh = _get_runner()
    # Per-core shard c is x[c*_BPC:(c+1)*_BPC].reshape(_ROWS, _COLS); stacking
    # the 8 shards along axis 0 is exactly the full tensor reshaped.
    g0 = np.ascontiguousarray(x0, dtype=np.float32).reshape(_NCORES * _ROWS, _COLS)
    g1 = np.ascontiguousarray(x1, dtype=np.float32).reshape(_NCORES * _ROWS, _COLS)
    (y,) = fn(g0, g1, zeros)
    return np.asarray(y).reshape(_B, _C, _H, _W)


# revision 14
# speedup vs baseline: 3722.1460x; 3722.1460x over previous
"""DWTFM fused kernel for Trainium2 (Bass/Tile), 8-core data parallel.

Math: the reference computes LL of dwt(x0), LH/HL/HH of dwt(x1), then idwt.
Algebraically this collapses to a local 2x2 stencil:

    out[2i+r, 2j+s] = x1[2i+r, 2j+s] + 0.25 * sum_{r',s'} (x0 - x1)[2i+r', 2j+s']

i.e. out = x1 + upsample2x2(blockmean2x2(x0 - x1)), independently per (b, c).

Layout: per core, the [2, 3, 512, 512] shard is viewed as [1536, 1024] where
each row is one "block-row" = two consecutive image rows concatenated
([even_row(512) | odd_row(512)]). 2x2 blocks never straddle rows of this view.
"""

import numpy as np

_B, _C, _H, _W = 16, 3, 512, 512
_NCORES = 8
_BPC = _B // _NCORES          # batch entries per core
_ROWS = _BPC * _C * _H // 2   # 1536 block-rows per core
_COLS = 2 * _W                # 1024
_P = 128                      # partitions per tile
_NT = _ROWS // _P             # 12 chunks per core


def _build(reps: int = 1):
    """Emit the Bass program. reps>1 repeats the full sweep back-to-back
    (same DRAM I/O) - used only for slope-based HW timing."""
    import concourse.bacc as bacc
    import concourse.mybir as mybir
    from concourse.tile import TileContext

    f32 = mybir.dt.float32
    W = _W
    J = W // 2  # 256 blocks per image row

    nc = bacc.Bacc("TRN2", target_bir_lowering=False)
    x0 = nc.dram_tensor("x0", [_ROWS, _COLS], f32, kind="ExternalInput").ap()
    x1 = nc.dram_tensor("x1", [_ROWS, _COLS], f32, kind="ExternalInput").ap()
    y = nc.dram_tensor("y", [_ROWS, _COLS], f32, kind="ExternalOutput").ap()

    with TileContext(nc) as tc:
        with tc.tile_pool(name="pool", bufs=3) as pool:
            for rep in range(reps):
                for k in range(_NT):
                    r = k * _P
                    t0 = pool.tile([_P, _COLS], f32, name="t0")
                    t1 = pool.tile([_P, _COLS], f32, name="t1")
                    nc.sync.dma_start(out=t0[:], in_=x0[r : r + _P, :])
                    nc.sync.dma_start(out=t1[:], in_=x1[r : r + _P, :])

                    # Vertical pair sums per input (each DVE op waits on only
                    # one DMA), then subtract:
                    # v = (x0_even + x0_odd) - (x1_even + x1_odd).
                    a = pool.tile([_P, W], f32, name="a")
                    nc.vector.tensor_add(
                        out=a[:], in0=t0[:, 0:W], in1=t0[:, W : 2 * W]
                    )
                    b = pool.tile([_P, W], f32, name="b")
                    nc.vector.tensor_add(
                        out=b[:], in0=t1[:, 0:W], in1=t1[:, W : 2 * W]
                    )
                    v = pool.tile([_P, W], f32, name="v")
                    nc.vector.tensor_sub(out=v[:], in0=a[:], in1=b[:])
                    # m[p, j] = v[p, 2j] + v[p, 2j+1]  (horizontal pair sum)
                    m = pool.tile([_P, J], f32, name="m")
                    v2 = v.rearrange("p (j s) -> p j s", s=2)
                    nc.vector.tensor_add(
                        out=m[:], in0=v2[:, :, 0], in1=v2[:, :, 1]
                    )

                    # mu[p, 2j+s] = 0.25 * m[p, j] (upsample + scale) on the
                    # otherwise-idle Scalar engine.
                    mu = pool.tile([_P, W], f32, name="mu")
                    mu3 = mu.rearrange("p (j s) -> p j s", s=2)
                    mb = m.unsqueeze(2).broadcast_to([_P, J, 2])
                    nc.scalar.activation(
                        mu3[:], mb, mybir.ActivationFunctionType.Copy, scale=0.25
                    )

                    # y = x1 + mu broadcast over the row-of-pair axis; single
                    # 3D TensorTensor so yt has one writer (keeps the store
                    # DMA's sync-wait count within ISA limits).
                    yt = pool.tile([_P, _COLS], f32, name="yt")
                    y3 = yt.rearrange("p (r2 w) -> p r2 w", r2=2)
                    x3 = t1.rearrange("p (r2 w) -> p r2 w", r2=2)
                    mub = mu.unsqueeze(1).broadcast_to([_P, 2, W])
                    nc.vector.tensor_add(out=y3[:], in0=x3[:], in1=mub)
                    nc.sync.dma_start(out=y[r : r + _P, :], in_=yt[:])
    nc.compile()
    return nc


def _make_runner(nc):
    """Jitted 8-core shard_map callable wrapping the Bass NEFF. Mirrors
    concourse.bass2jax.run_bass_via_pjrt but reusable across calls (no
    output-buffer donation, cached jit)."""
    import jax
    import concourse.mybir as mybir
    from concourse import bass2jax
    from jax.experimental.shard_map import shard_map
    from jax.sharding import Mesh, PartitionSpec

    bass2jax.install_neuronx_cc_hook()

    partition_name = (
        nc.partition_id_tensor.name if nc.partition_id_tensor else None
    )
    in_names = []
    out_names = []
    out_avals = []
    for alloc in nc.m.functions[0].allocations:
        if not isinstance(alloc, mybir.MemoryLocationSet):
            continue
        name = alloc.memorylocations[0].name
        if alloc.kind == "ExternalInput":
            if name != partition_name:
                in_names.append(name)
        elif alloc.kind == "ExternalOutput":
            out_names.append(name)
            out_avals.append(
                jax.core.ShapedArray(
                    tuple(alloc.tensor_shape), mybir.dt.np(alloc.dtype)
                )
            )
    assert in_names == ["x0", "x1"] and out_names == ["y"], (in_names, out_names)
    all_in_names = tuple(in_names + out_names)
    if partition_name is not None:
        all_in_names = all_in_names + (partition_name,)

    def _body(*args):
        operands = list(args)
        if partition_name is not None:
            operands.append(bass2jax.partition_id_tensor())
        outs = bass2jax._bass_exec_p.bind(
            *operands,
            out_avals=tuple(out_avals),
            in_names=all_in_names,
            out_names=tuple(out_names),
            lowering_input_output_aliases=(),
            sim_require_finite=True,
            sim_require_nnan=True,
            nc=nc,
        )
        return tuple(outs)

    devices = jax.devices()[:_NCORES]
    mesh = Mesh(np.asarray(devices), ("core",))
    n_args = len(in_names) + len(out_names)
    fn = jax.jit(
        shard_map(
            _body,
            mesh=mesh,
            in_specs=(PartitionSpec("core"),) * n_args,
            out_specs=(PartitionSpec("core"),) * len(out_names),
            check_rep=False,
        ),
        keep_unused=True,
    )
    return fn, mesh


_runners = {}


def get_runner(reps: int = 1):
    """(fn, zeros, mesh) for the reps-times-repeated sweep. reps=1 is the
    real kernel; other values exist for slope-based HW timing."""
    global _runners
    if reps not in _runners:
        import jax
        from jax.sharding import NamedSharding, PartitionSpec

        fn, mesh = _make_runner(_build(reps))
        zeros = jax.device_put(
            np.zeros((_NCORES * _ROWS, _COLS), np.float32),
            NamedSharding(mesh, PartitionSpec("core")),
        )
        _runners[reps] = (fn, zeros, mesh)
    return _runners[reps]


def kernel(x0: np.ndarray, x1: np.ndarray) -> np.ndarray:
    fn, zeros, _mesh = get_runner(1)
    # Per-core shard c is x[c*_BPC:(c+1)*_BPC].reshape(_ROWS, _COLS); stacking
    # the 8 shards along axis 0 is exactly the full tensor reshaped.
    g0 = np.ascontiguousarray(x0, dtype=np.float32).reshape(_NCORES * _ROWS, _COLS)
    g1 = np.ascontiguousarray(x1, dtype=np.float32).reshape(_NCORES * _ROWS, _COLS)
    (y,) = fn(g0, g1, zeros)
    return np.asarray(y).reshape(_B, _C, _H, _W)
